# revision 1
# baseline (speedup 1.0000x reference)
"""Trainium2 Bass kernel for nn_AttentionScore (sparse local attention scores).

Reference computation (B=4, C=64, N=16384, S=16):
    tmp   = xyz[:, :, :, None] - neighbor_xyz            # [B,3,N,S]
    pos   = concat([tmp, ||tmp||], axis=1)               # [B,4,N,S]
    k     = Wk @ (neighbor_points + Wpos @ pos + bpos)   # [B,C,N,S]
    attn  = softmax_s((points*scale) . k)                # [B,N,S]

Softmax over s is shift-invariant, so every term constant in s drops out:
    attn[m,s] ~ sum_c qW[c,m]*np[c,m,s] + sum_j qp[j,m]*tmp[j,m,s] + qp3[m]*||tmp||
with qW = (scale*Wk)^T @ points, qp = Wpos^T @ qW (bpos and the xyz.qp dot cancel).

Sharding: N split contiguously across 8 cores (no communication needed).
m = b*2048 + n_local in [0, 8192) per core, split in halves h = m // 4096.

Main-term dataflow per core:
  - np staged as [128 part = (h,c), (mm,s)] tiles; DVE multiplies by qW
    broadcast over s; TensorE reduces the 64 c-partitions per half with a
    block-ones [128,2] matmul (4x col-tiled into PSUM partitions 32j+h);
    ScalarE copies PSUM->SBUF; a partition-scatter SBUF->SBUF DMA lands
    results in the softmax layout [p = m//64, (m%64)*16+s].
"""

import os
import sys

sys.path.insert(0, "/opt/trn_rl_repo")

import numpy as np

import concourse.bass as bass
import concourse.bacc as bacc
import concourse.tile as tile
from concourse import mybir
from concourse.bass_utils import run_bass_kernel_spmd

F32 = mybir.dt.float32
F32R = mybir.dt.float32r
BF16 = mybir.dt.bfloat16
AF = mybir.ActivationFunctionType
AX = mybir.AxisListType
OP = mybir.AluOpType

B, C, N, S = 4, 64, 16384, 16
NCORES = 8
NL = N // NCORES            # 2048 points per core
M = B * NL                  # 8192 (b, n) rows per core
MH = M // 2                 # 4096 rows per half
MB = 256                    # mm per supertile (per half)
NT = MH // MB               # 16 supertiles
SCALE = float(C) ** -0.5

# float32r streams the moving operand at 1 col/cycle (vs 4 for fp32) and is
# more precise than the fp32 emulation path. Used for the big channel
# reduction only; small matmuls (qW, qp) stay fp32.
USE_F32R_REDUCE = True


def _body(tc):
    nc = tc.nc
    dma = nc.sync.dma_start

    NP = nc.dram_tensor("NP", [128, MH * S], F32, kind="ExternalInput").ap()
    NX = nc.dram_tensor("NX", [128, (M // 128) * 3 * S], F32, kind="ExternalInput").ap()
    XYZ = nc.dram_tensor("XYZ", [128, (M // 128) * 3], F32, kind="ExternalInput").ap()
    P = nc.dram_tensor("P", [C, M], F32, kind="ExternalInput").ap()
    WK = nc.dram_tensor("WK", [C, C], F32, kind="ExternalInput").ap()
    WKT = nc.dram_tensor("WKT", [C, C], F32, kind="ExternalInput").ap()
    WP = nc.dram_tensor("WP", [C, 4], F32, kind="ExternalInput").ap()
    OUT = nc.dram_tensor("OUT", [128, (M // 128) * S], F32, kind="ExternalOutput").ap()

    RDT = F32R if USE_F32R_REDUCE else F32

    with (
        tc.tile_pool(name="const", bufs=1) as cp,
        tc.tile_pool(name="w3072", bufs=2) as p3072,
        tc.tile_pool(name="w1024", bufs=3) as p1024,
        tc.tile_pool(name="small", bufs=1) as sp,
        # main-loop pools open up-front so NP prefetch DMAs have their SBUF
        # space from the start and fully overlap phase 1/2
        tc.tile_pool(name="npt", bufs=3) as npp,
        tc.tile_pool(name="prod", bufs=2) as prp,
        tc.tile_pool(name="sc", bufs=2) as scp,
        tc.tile_pool(name="psm", bufs=2, space="PSUM") as psm,
    ):
        # ---- constant loads ----
        wk = cp.tile([C, C], F32)
        dma(wk[:], WK)
        wkt = cp.tile([C, C], F32)
        dma(wkt[:], WKT)
        wp = cp.tile([C, 4], F32)
        dma(wp[:], WP)
        nxt = cp.tile([128, 64 * 3 * S], F32)
        dma(nxt[:], NX)
        xyzt = cp.tile([128, 64 * 3], F32)
        dma(xyzt[:], XYZ)

        wks = sp.tile([C, C], F32)
        nc.vector.tensor_scalar_mul(wks[:], wk[:], SCALE)
        wkts = sp.tile([C, C], F32)
        nc.vector.tensor_scalar_mul(wkts[:], wkt[:], SCALE)

        # Per-chunk half-selectors: lhsT for chunk k is hs[:, k*16:(k+1)*16],
        # whose column h*8+k is 1 on the h-half partitions. The 8 chunk
        # matmuls accumulate into one [16, 512] PSUM tile with chunk k's
        # half-h sums landing on row h*8+k (other rows accumulate zeros).
        # Built in f32 and copied with an f32r-rounding DVE op so the
        # fp32r matmul sees a properly "rounded" producer.
        hs0 = sp.tile([128, 16 * 8], F32)
        nc.vector.memset(hs0[:], 0.0)
        for k in range(8):
            nc.vector.memset(hs0[0:64, k * 16 + k:k * 16 + k + 1], 1.0)
            nc.vector.memset(hs0[64:128, k * 16 + 8 + k:k * 16 + 8 + k + 1], 1.0)
        hs = sp.tile([128, 16 * 8], RDT)
        nc.vector.tensor_copy(hs[:], hs0[:])

        qw = cp.tile([128, MH], F32)      # row h*64+c holds qW[c, h*MH + mm]
        qpt = cp.tile([128, 4 * 64], F32)  # row p, col j*64+mi: qp[j, p*64+mi]
        attn1 = cp.tile([128, 64 * S], F32)
        attn2 = cp.tile([128, 64 * S], F32)

        # ---- phase 1: qW / qp via bf16 Karatsuba on the PE ----
        # X @ Y ~= Xh@Yh + Xh@Yl + Xl@Yh with h/l the bf16 split; ~2^-18
        # relative error at 1 cycle/col (vs 4 for the fp32 emulation).
        # Chunked q keeps SBUF small so NP prefetch overlaps phase 1; (h0,h1)
        # chunk pairs emit in cc order so early supertiles unblock first.
        CH = 512
        NC1 = M // CH
        with (
            tc.tile_pool(name="qchunk", bufs=2) as qcp,
            tc.tile_pool(name="qps_p", bufs=2) as qpsp,
            tc.tile_pool(name="psq", bufs=2, space="PSUM") as psq,
            tc.tile_pool(name="psp", bufs=2, space="PSUM") as psp,
            tc.tile_pool(name="psw", bufs=1, space="PSUM") as psw,
        ):
            # Wkp[c, j] = sum_c' (scale*Wk)[c, c'] Wpos[c', j]  (fp32, tiny)
            pwkp = psw.tile([C, 4], F32)
            nc.tensor.matmul(pwkp[:], lhsT=wkts[:], rhs=wp[:], start=True, stop=True)
            wkp = sp.tile([C, 4], F32)
            nc.scalar.copy(wkp[:], pwkp[:])

            # Zero-padded qW weights: block h is [64, 128] with cols
            # h*64..h*64+64 = scale*Wk, so out rows h*64.. hold qW and every
            # PSUM partition is written. Split into bf16 hi/lo.
            wkh0 = sp.tile([C, 2 * 128], F32)
            nc.vector.memset(wkh0[:], 0.0)
            nc.vector.tensor_copy(wkh0[:, 0:64], wks[:])
            nc.vector.tensor_copy(wkh0[:, 192:256], wks[:])
            whh = sp.tile([C, 2 * 128], BF16)
            nc.vector.tensor_copy(whh[:], wkh0[:])
            whl0 = sp.tile([C, 2 * 128], F32)
            nc.vector.tensor_sub(whl0[:], wkh0[:], whh[:])
            whl = sp.tile([C, 2 * 128], BF16)
            nc.vector.tensor_copy(whl[:], whl0[:])

            wkph = sp.tile([C, 4], BF16)
            nc.vector.tensor_copy(wkph[:], wkp[:])
            wkpl0 = sp.tile([C, 4], F32)
            nc.vector.tensor_sub(wkpl0[:], wkp[:], wkph[:])
            wkpl = sp.tile([C, 4], BF16)
            nc.vector.tensor_copy(wkpl[:], wkpl0[:])

            # (h0, h1) chunk pairs in cc order so qw columns needed by the
            # first supertiles are produced first.
            qps_tiles = {}
            qps_fill = {}
            for cc in range(NC1 // 2):
              for h in range(2):
                t = h * (NC1 // 2) + cc
                rows = slice(h * 64, h * 64 + 64)
                wsl = slice(h * 128, (h + 1) * 128)

                qf = qcp.tile([C, CH], F32, tag="qf")
                # first pair rides the (empty) Sync queue ahead of NP tile 0
                # so supertile 0's qW dependency clears early; later chunks
                # go through SWDGE to keep Sync free for NP prefetch.
                if cc == 0:
                    dma(qf[:], P[:, t * CH:(t + 1) * CH])
                else:
                    nc.gpsimd.dma_start(qf[:], P[:, t * CH:(t + 1) * CH])
                qhh = qcp.tile([C, CH], BF16, tag="qhh")
                nc.scalar.copy(qhh[:], qf[:])
                qll = qcp.tile([C, CH], BF16, tag="qll")
                nc.vector.tensor_sub(qll[:], qf[:], qhh[:])

                cc2 = (t % (NC1 // 2)) * CH

                # qW[c', m] = sum_c (scale*Wk)[c, c'] q[c, m]
                pq = psq.tile([128, 512], F32)
                nc.tensor.matmul(pq[:], lhsT=whh[:, wsl], rhs=qhh[:], start=True, stop=False)
                nc.tensor.matmul(pq[:], lhsT=whh[:, wsl], rhs=qll[:], start=False, stop=False)
                nc.tensor.matmul(pq[:], lhsT=whl[:, wsl], rhs=qhh[:], start=False, stop=True)
                nc.scalar.copy(qw[rows, cc2:cc2 + CH], pq[rows, :])

                # qp[j, m] = sum_c Wkp[c, j] q[c, m]
                pp = psp.tile([4, 512], F32)
                nc.tensor.matmul(pp[:], lhsT=wkph[:], rhs=qhh[:], start=True, stop=False)
                nc.tensor.matmul(pp[:], lhsT=wkph[:], rhs=qll[:], start=False, stop=False)
                nc.tensor.matmul(pp[:], lhsT=wkpl[:], rhs=qhh[:], start=False, stop=True)

                g, gi = divmod(t, 4)
                if g not in qps_tiles:
                    qps_tiles[g] = qpsp.tile([4, 2048], F32, name="qps", tag="qps")
                    qps_fill[g] = 0
                qps = qps_tiles[g]
                nc.scalar.copy(qps[:, gi * 512:(gi + 1) * 512], pp[:])
                qps_fill[g] += 1
                if qps_fill[g] == 4:
                    # scatter qp group into softmax layout: qpt[p, j*64+mi]
                    for j in range(4):
                        nc.gpsimd.dma_start(
                            qpt[g * 32:(g + 1) * 32, j * 64:(j + 1) * 64],
                            qps[j:j + 1, :],
                        )
                    del qps_tiles[g]

        # ---- phase 2: positional term (whole core at once) ----
        # tmp[p, mi, j, s] = xyz[j, m] - nx[j, m, s]
        nx3 = nxt[:].rearrange("p (mi j s) -> p mi j s", mi=64, j=3, s=S)
        xyzb = (
            xyzt[:]
            .rearrange("p (mi j one) -> p mi j one", mi=64, j=3, one=1)
            .broadcast_to((128, 64, 3, S))
        )
        tmp = p3072.tile([128, 64 * 3 * S], F32, tag="big")
        tmp3 = tmp[:].rearrange("p (mi j s) -> p mi j s", mi=64, j=3, s=S)
        nc.vector.tensor_sub(tmp3, xyzb, nx3)

        sq = p3072.tile([128, 64 * 3 * S], F32, tag="big")
        nc.scalar.square(sq[:], tmp[:])

        norm2 = p1024.tile([128, 64 * S], F32, tag="w1k")
        nc.vector.reduce_sum(
            norm2[:].rearrange("p (mi s) -> p mi s", mi=64),
            sq[:].rearrange("p (mi j s) -> p mi s j", mi=64, j=3, s=S),
            axis=AX.X,
        )
        norm = p1024.tile([128, 64 * S], F32, tag="w1k")
        nc.scalar.sqrt(norm[:], norm2[:])

        # u = sum_j qp[j]*tmp[j]
        qptb3 = (
            qpt[:]
            .rearrange("p (j mi one) -> p mi j one", j=4, mi=64, one=1)[:, :, 0:3, :]
            .broadcast_to((128, 64, 3, S))
        )
        uw = p3072.tile([128, 64 * 3 * S], F32, tag="big")
        uw3 = uw[:].rearrange("p (mi j s) -> p mi j s", mi=64, j=3, s=S)
        nc.vector.tensor_mul(uw3, tmp3, qptb3)
        u = p1024.tile([128, 64 * S], F32, tag="w1k")
        nc.vector.reduce_sum(
            u[:].rearrange("p (mi s) -> p mi s", mi=64),
            uw[:].rearrange("p (mi j s) -> p mi s j", mi=64, j=3, s=S),
            axis=AX.X,
        )

        # attn2 = u + qp3 * norm
        qp3b = (
            qpt[:, 192:256]
            .rearrange("p (mi one) -> p mi one", one=1)
            .broadcast_to((128, 64, S))
        )
        a2 = p1024.tile([128, 64 * S], F32, tag="w1k")
        a23 = a2[:].rearrange("p (mi s) -> p mi s", mi=64)
        nc.vector.tensor_mul(a23, norm[:].rearrange("p (mi s) -> p mi s", mi=64), qp3b)
        nc.vector.tensor_add(attn2[:], a2[:], u[:])

        # ---- phase 3: main term supertiles ----
        if True:
            for t in range(NT):
                npt = npp.tile([128, MB * S], F32)
                dma(npt[:], NP[:, t * MB * S:(t + 1) * MB * S])

                prod = prp.tile([128, MB * S], RDT)
                qwb = (
                    qw[:, t * MB:(t + 1) * MB]
                    .rearrange("p (mm one) -> p mm one", one=1)
                    .broadcast_to((128, MB, S))
                )
                nc.vector.tensor_mul(
                    prod[:].rearrange("p (mm s) -> p mm s", s=S),
                    npt[:].rearrange("p (mm s) -> p mm s", s=S),
                    qwb,
                )

                ps = psm.tile([16, 512], F32)
                for k in range(8):
                    nc.tensor.matmul(
                        ps[:],
                        lhsT=hs[:, k * 16:(k + 1) * 16],
                        rhs=prod[:, k * 512:(k + 1) * 512],
                        start=(k == 0),
                        stop=(k == 7),
                    )
                sc = scp.tile([16, 512], F32)
                nc.scalar.copy(sc[:], ps[:])
                # row h*8+k holds chunk k / half h; lands at attn1 partition
                # h*64 + t*4 + k//2, cols (k%2)*512 + i*16 + s. Issued from
                # GPSIMD (SWDGE) so their waits don't stall the Sync queue
                # that prefetches NP tiles.
                for h in range(2):
                    nc.gpsimd.dma_start(
                        attn1[h * 64 + t * 4:h * 64 + t * 4 + 4, :].rearrange(
                            "p (k1 f) -> p k1 f", k1=2
                        ),
                        sc[h * 8:(h + 1) * 8, :],
                    )

        # ---- phase 4: softmax over s ----
        attn = p1024.tile([128, 64 * S], F32, tag="w1k")
        nc.vector.tensor_add(attn[:], attn1[:], attn2[:])
        at3 = attn[:].rearrange("p (mi s) -> p mi s", mi=64)

        mx = sp.tile([128, 64], F32)
        nc.vector.reduce_max(mx[:], at3, axis=AX.X)
        mxb = mx[:].rearrange("p (mi one) -> p mi one", one=1).broadcast_to((128, 64, S))
        xs = p1024.tile([128, 64 * S], F32, tag="w1k")
        nc.vector.tensor_sub(xs[:].rearrange("p (mi s) -> p mi s", mi=64), at3, mxb)

        e = p1024.tile([128, 64 * S], F32, tag="w1k")
        nc.scalar.activation(e[:], xs[:], AF.Exp)

        se = sp.tile([128, 64], F32)
        nc.vector.reduce_sum(se[:], e[:].rearrange("p (mi s) -> p mi s", mi=64), axis=AX.X)
        rse = sp.tile([128, 64], F32)
        nc.vector.reciprocal(rse[:], se[:])

        o = p1024.tile([128, 64 * S], F32, tag="w1k")
        rb = rse[:].rearrange("p (mi one) -> p mi one", one=1).broadcast_to((128, 64, S))
        nc.vector.tensor_mul(
            o[:].rearrange("p (mi s) -> p mi s", mi=64),
            e[:].rearrange("p (mi s) -> p mi s", mi=64),
            rb,
        )
        dma(OUT, o[:])


_NC_CACHE = None


def build_nc():
    global _NC_CACHE
    if _NC_CACHE is None:
        nc = bacc.Bacc(trn_type="TRN2", target_bir_lowering=False, debug=False)
        with tile.TileContext(nc) as tc:
            _body(tc)
        nc.compile()
        _NC_CACHE = nc
    return _NC_CACHE


def make_in_maps(xyz, neighbor_xyz, points, neighbor_points, Wk, Wpos, bpos):
    """Slice + relayout full inputs into the 8 per-core input maps."""
    xyz = np.asarray(xyz, dtype=np.float32)
    neighbor_xyz = np.asarray(neighbor_xyz, dtype=np.float32)
    points = np.asarray(points, dtype=np.float32)
    neighbor_points = np.asarray(neighbor_points, dtype=np.float32)
    Wk = np.ascontiguousarray(np.asarray(Wk, dtype=np.float32))
    WkT = np.ascontiguousarray(Wk.T)
    Wp = np.ascontiguousarray(np.asarray(Wpos, dtype=np.float32))

    in_maps = []
    for i in range(NCORES):
        nsl = slice(i * NL, (i + 1) * NL)
        # np: [B,C,nl,S] -> [c, m, s] -> [h, c, mm, s] -> [128, MH*S]
        npc = neighbor_points[:, :, nsl, :].transpose(1, 0, 2, 3).reshape(C, M, S)
        npc = (
            npc.reshape(C, 2, MH, S).transpose(1, 0, 2, 3).reshape(128, MH * S)
        )
        # nx: [B,3,nl,S] -> [m, j, s] -> [128, 64*3*S]
        nxc = (
            neighbor_xyz[:, :, nsl, :]
            .transpose(1, 0, 2, 3)
            .reshape(3, M, S)
            .transpose(1, 0, 2)
            .reshape(128, 64 * 3 * S)
        )
        # xyz: [B,3,nl] -> [m, j] -> [128, 192]
        xc = (
            xyz[:, :, nsl]
            .transpose(1, 0, 2)
            .reshape(3, M)
            .T.reshape(128, 64 * 3)
        )
        # points: [B,C,nl] -> [c, m]
        pc = points[:, :, nsl].transpose(1, 0, 2).reshape(C, M)
        in_maps.append(
            {
                "NP": np.ascontiguousarray(npc),
                "NX": np.ascontiguousarray(nxc),
                "XYZ": np.ascontiguousarray(xc),
                "P": np.ascontiguousarray(pc),
                "WK": Wk,
                "WKT": WkT,
                "WP": Wp,
            }
        )
    return in_maps


def assemble_output(results):
    """Per-core OUT [128, 64*S] -> full [B, N, S]."""
    out = np.empty((B, N, S), dtype=np.float32)
    for i in range(NCORES):
        oc = np.asarray(results[i]["OUT"]).reshape(M, S)  # m = p*64+mi row-major
        out[:, i * NL:(i + 1) * NL, :] = oc.reshape(B, NL, S)
    return out


def run_cores(in_maps, trace=False, trace_kwargs=None):
    nc = build_nc()
    return run_bass_kernel_spmd(
        nc,
        in_maps,
        core_ids=list(range(NCORES)),
        trace=trace,
        **(trace_kwargs or {}),
    )


def kernel(xyz, neighbor_xyz, points, neighbor_points, Wk, Wpos, bpos):
    in_maps = make_in_maps(
        xyz, neighbor_xyz, points, neighbor_points, Wk, Wpos, bpos
    )
    res = run_cores(in_maps, trace=False)
    return assemble_output(res.results)



# revision 3
# speedup vs baseline: 1.6920x; 1.6920x over previous
"""Trainium2 Bass kernel for nn_AttentionScore (sparse local attention scores).

Reference computation (B=4, C=64, N=16384, S=16):
    tmp   = xyz[:, :, :, None] - neighbor_xyz            # [B,3,N,S]
    pos   = concat([tmp, ||tmp||], axis=1)               # [B,4,N,S]
    k     = Wk @ (neighbor_points + Wpos @ pos + bpos)   # [B,C,N,S]
    attn  = softmax_s((points*scale) . k)                # [B,N,S]

Softmax over s is shift-invariant, so every term constant in s drops out:
    attn[m,s] ~ sum_c qW[c,m]*np[c,m,s] - sum_j qp[j,m]*nx[j,m,s]
                + qp3[m]*sqrt(max(0, ||xyz||^2 + sum_j (nx[j]-2*xyz[j])*nx[j]))
with qW = (scale*Wk)^T @ points, qp = ((scale*Wk)@Wpos)^T @ points
(bpos and all xyz-only dot products cancel).

All bulk tensors are pre-cast to bf16 on the host (tolerance is 2e-2;
measured end-to-end error of the bf16 pipeline is ~3e-3), halving HBM
traffic and enabling the DVE 2x bf16 tensor_tensor mode.

Sharding: N split contiguously across 8 cores (no communication).
m = b*2048 + n_local in [0, 8192) per core, halves h = m // 4096.

Main-term dataflow per core, per supertile t (8 supertiles, 512 m per half):
  np staged bf16 as [128 part = (h,c), cols (mh:8, s:16, ml:64)];
  DVE multiplies by qW broadcast over s (dense innermost ml run keeps the
  2x bf16 mode); TensorE reduces the 64 c-partitions per half with
  selector-column matmuls accumulating into one [32, 512] PSUM tile
  (row h*16 + (mh*2+sh)); ScalarE copies PSUM->SBUF; a partition-scatter
  SBUF->SBUF DMA lands rows in the softmax layout [p=m//64, (s:16, mi:64)].
"""

import sys

sys.path.insert(0, "/opt/trn_rl_repo")

import numpy as np
import ml_dtypes

import concourse.bass as bass
import concourse.bacc as bacc
import concourse.tile as tile
from concourse import mybir
from concourse.bass_utils import run_bass_kernel_spmd

F32 = mybir.dt.float32
BF16 = mybir.dt.bfloat16
AF = mybir.ActivationFunctionType
AX = mybir.AxisListType
NPBF = ml_dtypes.bfloat16

B, C, N, S = 4, 64, 16384, 16
NCORES = 8
NL = N // NCORES            # 2048 points per core
M = B * NL                  # 8192 (b, n) rows per core
MH = M // 2                 # 4096 rows per half
NT = 8                      # supertiles
MB = MH // NT               # 512 m per half per supertile
SCALE = float(C) ** -0.5

# per-supertile NP tile: [128 p=(h,c), cols (mh:8, s:16, ml:64)] = 8192 cols
TS = MB * S                 # 8192 columns per supertile


def _body(tc):
    nc = tc.nc
    dma = nc.sync.dma_start
    gdma = nc.gpsimd.dma_start

    NP = nc.dram_tensor("NP", [128, NT * TS], BF16, kind="ExternalInput").ap()
    NX = nc.dram_tensor("NX", [128, 3 * S * 64], BF16, kind="ExternalInput").ap()
    XYZ = nc.dram_tensor("XYZ", [128, 3 * 64], F32, kind="ExternalInput").ap()
    P = nc.dram_tensor("P", [C, M], BF16, kind="ExternalInput").ap()
    WK = nc.dram_tensor("WK", [C, C], F32, kind="ExternalInput").ap()
    WKT = nc.dram_tensor("WKT", [C, C], F32, kind="ExternalInput").ap()
    WP = nc.dram_tensor("WP", [C, 4], F32, kind="ExternalInput").ap()
    OUT = nc.dram_tensor("OUT", [128, 64 * S], F32, kind="ExternalOutput").ap()

    with (
        tc.tile_pool(name="const", bufs=1) as cp,
        tc.tile_pool(name="small", bufs=1) as sp,
        tc.tile_pool(name="work", bufs=2) as wkp_,
        tc.tile_pool(name="npt", bufs=3) as npp,
        tc.tile_pool(name="prod", bufs=2) as prp,
        tc.tile_pool(name="sc", bufs=2) as scp,
        tc.tile_pool(name="psm", bufs=2, space="PSUM") as psm,
    ):
        # ---- constant loads (gpsimd queue; sync queue reserved for NP) ----
        wk0 = cp.tile([C, C], F32)
        gdma(wk0[:], WK)
        wkt0 = cp.tile([C, C], F32)
        gdma(wkt0[:], WKT)
        wp0 = cp.tile([C, 4], F32)
        gdma(wp0[:], WP)

        # scaled weights
        wkb = sp.tile([C, C], BF16)
        nc.vector.tensor_scalar_mul(wkb[:], wk0[:], SCALE)
        wkts = sp.tile([C, C], F32)
        nc.vector.tensor_scalar_mul(wkts[:], wkt0[:], SCALE)

        # Selector for the channel-reduce matmuls: chunk k uses cols
        # [k*32, (k+1)*32); col h*16+k is 1 on the half-h partitions, so
        # chunk k's half-h sum lands on PSUM row h*16+k.
        hs = sp.tile([128, 16 * 32], BF16)
        nc.vector.memset(hs[:], 0.0)
        for k in range(16):
            nc.vector.memset(hs[0:64, k * 32 + k:k * 32 + k + 1], 1.0)
            nc.vector.memset(hs[64:128, k * 32 + 16 + k:k * 32 + 16 + k + 1], 1.0)

        qw = cp.tile([128, MH], BF16)        # row h*64+c: qW[c, h*MH + mm]
        qpt = cp.tile([128, 4 * 64], BF16)   # [p=m//64, (j:4, mi:64)]
        attn1 = cp.tile([128, 64 * S], F32)  # [p=m//64, (s:16, mi:64)]
        attn2 = cp.tile([128, 64 * S], F32)

        # ---- phase 1: qW = (sWk)^T q, qp = ((sWk)Wpos)^T q, bf16 on PE ----
        with (
            tc.tile_pool(name="qchunk", bufs=4) as qcp,
            tc.tile_pool(name="qps_p", bufs=2) as qpsp,
            tc.tile_pool(name="psq", bufs=2, space="PSUM") as psq,
            tc.tile_pool(name="psp", bufs=2, space="PSUM") as psp,
            tc.tile_pool(name="psw", bufs=1, space="PSUM") as psw,
        ):
            # Wkp[c, j] = sum_c' sWk[c, c'] Wpos[c', j]  (fp32, tiny)
            pwkp = psw.tile([C, 4], F32)
            nc.tensor.matmul(pwkp[:], lhsT=wkts[:], rhs=wp0[:], start=True, stop=True)
            wkpb = sp.tile([C, 4], BF16)
            nc.scalar.copy(wkpb[:], pwkp[:])

            qps_tiles = {}
            qps_fill = {}
            for cc in range(8):
                for h in range(2):
                    qf = qcp.tile([C, 512], BF16, tag="qf")
                    gdma(qf[:], P[:, h * MH + cc * 512:h * MH + (cc + 1) * 512])

                    # qW chunk: rows c' for half h
                    pq = psq.tile([C, 512], F32)
                    nc.tensor.matmul(pq[:], lhsT=wkb[:], rhs=qf[:], start=True, stop=True)
                    nc.scalar.copy(qw[h * 64:(h + 1) * 64, cc * 512:(cc + 1) * 512], pq[:])

                    # qp chunk
                    pp = psp.tile([4, 512], F32)
                    nc.tensor.matmul(pp[:], lhsT=wkpb[:], rhs=qf[:], start=True, stop=True)
                    g = h * 2 + cc // 4
                    if g not in qps_tiles:
                        qps_tiles[g] = qpsp.tile([4, 2048], BF16, name="qps", tag="qps")
                        qps_fill[g] = 0
                    qps = qps_tiles[g]
                    nc.scalar.copy(qps[:, (cc % 4) * 512:(cc % 4 + 1) * 512], pp[:])
                    qps_fill[g] += 1
                    if qps_fill[g] == 4:
                        # scatter into softmax layout: qpt[p, j*64+mi]
                        for j in range(4):
                            gdma(
                                qpt[g * 32:(g + 1) * 32, j * 64:(j + 1) * 64],
                                qps[j:j + 1, :],
                            )
                        del qps_tiles[g]

        # phase-2 inputs arrive on the gpsimd queue after the qf chunks
        nxt = cp.tile([128, 3 * S * 64], BF16)
        gdma(nxt[:], NX)
        xyzt = cp.tile([128, 3 * 64], F32)
        gdma(xyzt[:], XYZ)

        def phase2():
            # xyz-derived constants
            xyz2 = sp.tile([128, 3 * 64], BF16)      # -2*xyz
            nc.vector.tensor_scalar_mul(xyz2[:], xyzt[:], -2.0)
            xsq = sp.tile([128, 3 * 64], F32)
            nc.scalar.square(xsq[:], xyzt[:])
            x2a = sp.tile([128, 64], F32)
            nc.vector.tensor_add(x2a[:], xsq[:, 0:64], xsq[:, 64:128])
            x2s = sp.tile([128, 64], BF16)           # ||xyz||^2 per m
            nc.vector.tensor_add(x2s[:], x2a[:], xsq[:, 128:192])

            nx4 = nxt[:].rearrange("p (j s mi) -> p j s mi", j=3, s=S, mi=64)
            # d = nx - 2*xyz
            dt_ = wkp_.tile([128, 3 * S * 64], BF16, tag="w3k")
            d4 = dt_[:].rearrange("p (j s mi) -> p j s mi", j=3, s=S, mi=64)
            xyz2b = (
                xyz2[:]
                .rearrange("p (j one mi) -> p j one mi", j=3, one=1, mi=64)
                .broadcast_to((128, 3, S, 64))
            )
            nc.vector.tensor_add(d4, nx4, xyz2b)
            # cxsq = nx * d ; norm2 = sum_j cxsq + ||xyz||^2
            cs = wkp_.tile([128, 3 * S * 64], BF16, tag="w3k")
            cs4 = cs[:].rearrange("p (j s mi) -> p j s mi", j=3, s=S, mi=64)
            nc.vector.tensor_mul(cs4, nx4, d4)
            n2a = wkp_.tile([128, 64 * S], BF16, tag="w1k")
            nc.vector.tensor_add(n2a[:], cs[:, 0:1024], cs[:, 1024:2048])
            n2b = wkp_.tile([128, 64 * S], BF16, tag="w1k")
            nc.vector.tensor_add(n2b[:], n2a[:], cs[:, 2048:3072])
            x2sb = (
                x2s[:]
                .rearrange("p (one mi) -> p one mi", one=1)
                .broadcast_to((128, S, 64))
            )
            n2c = wkp_.tile([128, 64 * S], BF16, tag="w1k")
            nc.vector.tensor_add(
                n2c[:].rearrange("p (s mi) -> p s mi", s=S),
                n2b[:].rearrange("p (s mi) -> p s mi", s=S),
                x2sb,
            )
            n2d = wkp_.tile([128, 64 * S], BF16, tag="w1k")
            nc.vector.tensor_scalar_max(n2d[:], n2c[:], 0.0)
            nrm = wkp_.tile([128, 64 * S], BF16, tag="nrm")
            nc.scalar.sqrt(nrm[:], n2d[:])

            # pl = nx * qp ; pls = sum_j pl
            qpb = (
                qpt[:, 0:192]
                .rearrange("p (j one mi) -> p j one mi", j=3, one=1, mi=64)
                .broadcast_to((128, 3, S, 64))
            )
            pl = wkp_.tile([128, 3 * S * 64], BF16, tag="w3k")
            pl4 = pl[:].rearrange("p (j s mi) -> p j s mi", j=3, s=S, mi=64)
            nc.vector.tensor_mul(pl4, nx4, qpb)
            pla = wkp_.tile([128, 64 * S], BF16, tag="w1k")
            nc.vector.tensor_add(pla[:], pl[:, 0:1024], pl[:, 1024:2048])
            plb = wkp_.tile([128, 64 * S], BF16, tag="w1k")
            nc.vector.tensor_add(plb[:], pla[:], pl[:, 2048:3072])

            # attn2 = qp3*norm - pls  (fp32)
            qp3b = (
                qpt[:, 192:256]
                .rearrange("p (one mi) -> p one mi", one=1)
                .broadcast_to((128, S, 64))
            )
            a2m = wkp_.tile([128, 64 * S], F32, tag="w1kf")
            nc.vector.tensor_mul(
                a2m[:].rearrange("p (s mi) -> p s mi", s=S),
                nrm[:].rearrange("p (s mi) -> p s mi", s=S),
                qp3b,
            )
            plf = wkp_.tile([128, 64 * S], F32, tag="w1kf")
            nc.vector.tensor_copy(plf[:], plb[:])
            nc.vector.tensor_sub(attn2[:], a2m[:], plf[:])

        # ---- phase 3: main-term supertiles ----
        for t in range(NT):
            npt = npp.tile([128, TS], BF16)
            dma(npt[:], NP[:, t * TS:(t + 1) * TS])

            prod = prp.tile([128, TS], BF16)
            qwb = (
                qw[:, t * 512:(t + 1) * 512]
                .rearrange("p (mh one ml) -> p mh one ml", mh=8, one=1, ml=64)
                .broadcast_to((128, 8, S, 64))
            )
            nc.vector.tensor_mul(
                prod[:].rearrange("p (mh s ml) -> p mh s ml", mh=8, s=S, ml=64),
                npt[:].rearrange("p (mh s ml) -> p mh s ml", mh=8, s=S, ml=64),
                qwb,
            )

            ps = psm.tile([32, 512], F32)
            for k in range(16):
                nc.tensor.matmul(
                    ps[:],
                    lhsT=hs[:, k * 32:(k + 1) * 32],
                    rhs=prod[:, k * 512:(k + 1) * 512],
                    start=(k == 0),
                    stop=(k == 15),
                )
            sc = scp.tile([32, 512], F32)
            nc.scalar.copy(sc[:], ps[:])
            # row h*16 + (mh*2+sh) holds cols (s8:8, ml:64) of dst partition
            # h*64 + t*8 + mh, col (sh*8+s8)*64 + ml.
            for h in range(2):
                gdma(
                    attn1[h * 64 + t * 8:h * 64 + t * 8 + 8, :].rearrange(
                        "p (sh s8 ml) -> p sh s8 ml", sh=2, s8=8, ml=64
                    ),
                    sc[h * 16:(h + 1) * 16, :],
                )
            if t == 1:
                phase2()

        # ---- phase 4: softmax over s (no max-sub; |attn| < 4) ----
        attn = wkp_.tile([128, 64 * S], F32, tag="w1kf")
        nc.vector.tensor_add(attn[:], attn1[:], attn2[:])
        e = wkp_.tile([128, 64 * S], F32, tag="e")
        nc.scalar.activation(e[:], attn[:], AF.Exp)
        se = sp.tile([128, 64], F32)
        nc.vector.reduce_sum(
            se[:], e[:].rearrange("p (s mi) -> p mi s", s=S), axis=AX.X
        )
        rse = sp.tile([128, 64], F32)
        nc.vector.reciprocal(rse[:], se[:])
        o = wkp_.tile([128, 64 * S], F32, tag="w1kf")
        rb = rse[:].rearrange("p (one mi) -> p one mi", one=1).broadcast_to((128, S, 64))
        nc.vector.tensor_mul(
            o[:].rearrange("p (s mi) -> p s mi", s=S),
            e[:].rearrange("p (s mi) -> p s mi", s=S),
            rb,
        )
        dma(OUT, o[:])


_NC_CACHE = None


def build_nc():
    global _NC_CACHE
    if _NC_CACHE is None:
        nc = bacc.Bacc(trn_type="TRN2", target_bir_lowering=False, debug=False)
        with tile.TileContext(nc) as tc:
            _body(tc)
        nc.compile()
        _NC_CACHE = nc
    return _NC_CACHE


def make_in_maps(xyz, neighbor_xyz, points, neighbor_points, Wk, Wpos, bpos):
    """Slice + relayout + bf16-cast full inputs into the 8 per-core maps."""
    xyz = np.asarray(xyz, dtype=np.float32)
    neighbor_xyz = np.asarray(neighbor_xyz, dtype=np.float32)
    points = np.asarray(points, dtype=np.float32)
    neighbor_points = np.asarray(neighbor_points, dtype=np.float32)
    Wk32 = np.ascontiguousarray(np.asarray(Wk, dtype=np.float32))
    WkT = np.ascontiguousarray(Wk32.T)
    Wp32 = np.ascontiguousarray(np.asarray(Wpos, dtype=np.float32))

    in_maps = []
    for i in range(NCORES):
        nsl = slice(i * NL, (i + 1) * NL)
        # np: [B,C,nl,S] -> [c,m,s] -> [(h,c), (t, mh, s, ml)]
        npc = neighbor_points[:, :, nsl, :].transpose(1, 0, 2, 3).reshape(C, M, S)
        npd = (
            npc.reshape(C, 2, NT, 8, 64, S)
            .transpose(1, 0, 2, 3, 5, 4)
            .reshape(128, NT * TS)
        )
        # nx: [B,3,nl,S] -> [j,m,s] -> [p=m//64, (j, s, mi)]
        nxc = (
            neighbor_xyz[:, :, nsl, :]
            .transpose(1, 0, 2, 3)
            .reshape(3, M, S)
            .reshape(3, 128, 64, S)
            .transpose(1, 0, 3, 2)
            .reshape(128, 3 * S * 64)
        )
        # xyz: [B,3,nl] -> [p, (j, mi)]
        xc = (
            xyz[:, :, nsl]
            .transpose(1, 0, 2)
            .reshape(3, 128, 64)
            .transpose(1, 0, 2)
            .reshape(128, 192)
        )
        # points: [c, m]
        pc = points[:, :, nsl].transpose(1, 0, 2).reshape(C, M)
        in_maps.append(
            {
                "NP": np.ascontiguousarray(npd.astype(NPBF)),
                "NX": np.ascontiguousarray(nxc.astype(NPBF)),
                "XYZ": np.ascontiguousarray(xc),
                "P": np.ascontiguousarray(pc.astype(NPBF)),
                "WK": Wk32,
                "WKT": WkT,
                "WP": Wp32,
            }
        )
    return in_maps


def assemble_output(results):
    """Per-core OUT [128, (s:16, mi:64)] -> full [B, N, S]."""
    out = np.empty((B, N, S), dtype=np.float32)
    for i in range(NCORES):
        oc = np.asarray(results[i]["OUT"]).reshape(128, S, 64)
        oc = oc.transpose(0, 2, 1).reshape(M, S)
        out[:, i * NL:(i + 1) * NL, :] = oc.reshape(B, NL, S)
    return out


def run_cores(in_maps, trace=False, trace_kwargs=None):
    nc = build_nc()
    return run_bass_kernel_spmd(
        nc,
        in_maps,
        core_ids=list(range(NCORES)),
        trace=trace,
        **(trace_kwargs or {}),
    )


def kernel(xyz, neighbor_xyz, points, neighbor_points, Wk, Wpos, bpos):
    in_maps = make_in_maps(
        xyz, neighbor_xyz, points, neighbor_points, Wk, Wpos, bpos
    )
    res = run_cores(in_maps, trace=False)
    return assemble_output(res.results)
